# revision 1
# baseline (speedup 1.0000x reference)
"""Multi-head attention forward on 8 TRN2 NeuronCores.

Problem: x[2,2048,1024] @ {Wq,Wk,Wv}[1024,1024] (+bias) -> 16 heads of 64,
softmax(QK^T/8)V per head, concat -> @Wo[1024,1024] + bo.

Sharding: tensor-parallel over d_hid. Core c owns 2 heads (128 dims):
  - computes Q^T,K^T,V^T slices [128, 4096] from full x^T
  - attention for its (2 batches x 2 heads)
  - partial out = ctx_slice @ Wo[slice_rows] -> [4096, 1024]
Host sums the 8 partials and adds bo (pure reduction, no collectives).

Device layout notes:
  - x^T [1024, 4096] uploaded pre-transposed (host prep) so the contraction
    dim (d_in) lands on SBUF partitions for all projection matmuls.
  - Wq, bq pre-scaled by 1/8 on host (folds the softmax scale).
  - All matmuls run in float32r (fp32 single-pass mode, ~1e-4 rel err).
  - Scores computed transposed (S^T[k,q]) so softmax normalization comes
    from a ones-column augmented V (row 64 of the ctx psum = row sums).
  - PSUM banks: scores h0/h1 double-buffered (4) + ctx accum h0/h1 (2) +
    proj/transpose/outproj slots p0/p1 (2) = 8.
"""

import os
import numpy as np

B, S, D = 2, 2048, 1024
NCORES = 8
HSLICE = D // NCORES          # 128 = 2 heads x 64
KT_PROJ = D // 128            # 8 contraction tiles for projections
NKT = S // 128                # 16 k-tiles per batch for attention
QH = 512                      # q chunk (1 PSUM bank)
CH = 512                      # matmul free-dim chunk

_cache = {}


def _build():
    import concourse.bacc as bacc
    import concourse.tile as tile
    from concourse import mybir
    from concourse.tile_rust import add_dep_helper

    f32 = mybir.dt.float32
    f32r = mybir.dt.float32r
    f16 = mybir.dt.float16
    AF = mybir.ActivationFunctionType

    nc = bacc.Bacc("TRN2", target_bir_lowering=False, debug=False,
                   num_devices=NCORES)

    xt_d = nc.dram_tensor("xt", [D, B * S], f16, kind="ExternalInput").ap()
    wq_d = nc.dram_tensor("wq", [D, HSLICE], f16, kind="ExternalInput").ap()
    wk_d = nc.dram_tensor("wk", [D, HSLICE], f16, kind="ExternalInput").ap()
    wv_d = nc.dram_tensor("wv", [D, HSLICE], f16, kind="ExternalInput").ap()
    bq_d = nc.dram_tensor("bq", [HSLICE, 1], f32, kind="ExternalInput").ap()
    bk_d = nc.dram_tensor("bk", [HSLICE, 1], f32, kind="ExternalInput").ap()
    bv_d = nc.dram_tensor("bv", [HSLICE, 1], f32, kind="ExternalInput").ap()
    wo_d = nc.dram_tensor("wo", [HSLICE, D], f32r, kind="ExternalInput").ap()
    idt_d = nc.dram_tensor("idt", [128, 128], f32r, kind="ExternalInput").ap()
    ones_d = nc.dram_tensor("ones", [128, 1], f16, kind="ExternalInput").ap()
    out_d = nc.dram_tensor("out", [B * S, D], f32, kind="ExternalOutput").ap()

    with tile.TileContext(nc) as tc:
        with (
            tc.tile_pool(name="wpool", bufs=1) as wpool,
            tc.tile_pool(name="xt", bufs=1) as xtp,
            tc.tile_pool(name="qk", bufs=2) as qkp,
            tc.tile_pool(name="vtmp", bufs=1) as vtp,
            tc.tile_pool(name="vaug", bufs=2) as vap,
            tc.tile_pool(name="et", bufs=2) as etp,
            tc.tile_pool(name="ctx", bufs=2) as ctxp,
            tc.tile_pool(name="norm", bufs=1) as normp,
            tc.tile_pool(name="ost", bufs=3) as ostp,
            tc.tile_pool(name="psS", bufs=1, space="PSUM") as psS,
            tc.tile_pool(name="psC", bufs=1, space="PSUM") as psC,
            tc.tile_pool(name="psP", bufs=2, space="PSUM") as psP,
        ):
            # ---- constants / weights ----
            wq_t, wk_t, wv_t = [], [], []
            for ki in range(KT_PROJ):
                for lst, src, tag in ((wq_t, wq_d, "wq"), (wk_t, wk_d, "wk"),
                                      (wv_t, wv_d, "wv")):
                    t = wpool.tile([128, HSLICE], f16, tag=f"{tag}{ki}")
                    nc.scalar.dma_start(t[:], src[ki * 128:(ki + 1) * 128, :])
                    lst.append(t)
            wo_t = wpool.tile([128, D], f32r, tag="wo")
            nc.scalar.dma_start(wo_t[:], wo_d[:])
            idt = wpool.tile([128, 128], f32r, tag="idt")
            nc.scalar.dma_start(idt[:], idt_d[:])
            ones_t = wpool.tile([128, 1], f16, tag="ones")
            nc.scalar.dma_start(ones_t[:], ones_d[:])
            bq_t = wpool.tile([128, 1], f32, tag="bq")
            nc.scalar.dma_start(bq_t[:], bq_d[:])
            bk_t = wpool.tile([128, 1], f32, tag="bk")
            nc.scalar.dma_start(bk_t[:], bk_d[:])
            bv_t = wpool.tile([128, 1], f32, tag="bv")
            nc.scalar.dma_start(bv_t[:], bv_d[:])

            for b in range(B):
                s0 = b * S
                # ---- load x^T for this batch, column-sliced so the first
                # projection chunk can start after ~1/4 of the data ----
                xts = []
                for ki in range(KT_PROJ):
                    t = xtp.tile([128, S], f16, tag=f"xt{ki}")
                    xts.append(t)
                for c in range(S // CH):
                    for ki in range(KT_PROJ):
                        nc.sync.dma_start(
                            xts[ki][:, c * CH:(c + 1) * CH],
                            xt_d[ki * 128:(ki + 1) * 128,
                                 s0 + c * CH:s0 + (c + 1) * CH])

                # ---- projections: Q^T (split per head, zero-padded), K^T,
                # V^T [128, 2048]. qth[h] has the other head's 64 rows zeroed
                # so scores can run full-K=128 matmuls (keeps the PE array
                # fully active -> HAM stays un-throttled).
                qt0 = qkp.tile([128, S], f16, tag="qt0")
                qt1 = qkp.tile([128, S], f16, tag="qt1")
                qth = [qt0, qt1]
                nc.vector.memset(qt0[64:128, :], 0.0)
                nc.vector.memset(qt1[0:64, :], 0.0)
                kt = qkp.tile([128, S], f16, tag="kt")
                vt = vtp.tile([128, S], f32r, tag="vt")
                for di, (dst, w_t, b_t) in enumerate(
                        ((None, wq_t, bq_t), (kt, wk_t, bk_t), (vt, wv_t, bv_t))):
                    for c in range(S // CH):
                        ps = psP.tile([128, CH], f32, tag="pp")
                        for ki in range(KT_PROJ):
                            nc.tensor.matmul(ps[:], w_t[ki][:],
                                             xts[ki][:, c * CH:(c + 1) * CH],
                                             start=(ki == 0),
                                             stop=(ki == KT_PROJ - 1))
                        if dst is None:
                            nc.vector.tensor_scalar_add(
                                qt0[0:64, c * CH:(c + 1) * CH],
                                ps[0:64, :], b_t[0:64, 0:1])
                            nc.vector.tensor_scalar_add(
                                qt1[64:128, c * CH:(c + 1) * CH],
                                ps[64:128, :], b_t[64:128, 0:1])
                        else:
                            nc.vector.tensor_scalar_add(
                                dst[:, c * CH:(c + 1) * CH], ps[:], b_t[:, 0:1])

                # ---- V^T -> V_aug tiles [128, 130] (ones at cols 64, 129) ----
                vaugs = []
                for ki in range(NKT):
                    va = vap.tile([128, 130], f16, tag=f"va{ki}")
                    ps = psP.tile([128, 128], f32r, tag="pp")
                    nc.tensor.transpose(ps[:], vt[:, ki * 128:(ki + 1) * 128],
                                        idt[:])
                    nc.vector.tensor_copy(va[:, 0:64], ps[:, 0:64])
                    nc.vector.tensor_copy(va[:, 65:129], ps[:, 64:128])
                    nc.vector.tensor_copy(va[:, 64:65], ones_t[:])
                    nc.vector.tensor_copy(va[:, 129:130], ones_t[:])
                    vaugs.append(va)

                # ---- attention: both heads interleaved (keeps PE dense) ----
                ctxT = ctxp.tile([128, S], f32r, tag="ctxT")
                for qh in range(S // QH):
                    q0 = qh * QH
                    ctx_ps0 = psC.tile([65, QH], f32, tag="ctx0")
                    ctx_ps1 = psC.tile([65, QH], f32, tag="ctx1")
                    ctx_ps = [ctx_ps0, ctx_ps1]

                    def ctx_step(kp, ets):
                        for h in range(2):
                            for j in range(2):
                                ki = 2 * kp + j
                                nc.tensor.matmul(
                                    ctx_ps[h][:],
                                    vaugs[ki][:, h * 65:h * 65 + 65],
                                    ets[h][:, j * QH:(j + 1) * QH],
                                    start=(ki == 0), stop=(ki == NKT - 1))

                    # software pipeline: score pair [ki] runs back-to-back
                    # (row-group concurrent), ctx pair [ki-1] fills the exp
                    # latency.
                    prev = None
                    for kp in range(NKT // 2):
                        scs, ets = [], []
                        for h in range(2):
                            sc = psS.tile([128, 2 * QH], f32, tag=f"sc{h}")
                            for j in range(2):
                                ki = 2 * kp + j
                                nc.tensor.matmul(
                                    sc[:, j * QH:(j + 1) * QH],
                                    kt[:, ki * 128:(ki + 1) * 128],
                                    qth[h][:, q0:q0 + QH])
                            scs.append(sc)
                        for h in range(2):
                            et = etp.tile([128, 2 * QH], f16, tag=f"et{h}")
                            nc.scalar.activation(et[:], scs[h][:], AF.Exp)
                            ets.append(et)
                        if prev is not None:
                            ctx_step(prev[0], prev[1])
                        prev = (kp, ets)
                    ctx_step(prev[0], prev[1])
                    # normalize: stage psum (data + sums row 64) to SBUF in
                    # one copy so the ctx bank frees immediately, then
                    # normalize entirely from SBUF off the critical path.
                    for h in range(2):
                        hp = h * 64
                        stg = normp.tile([128, QH], f32, tag=f"stg{h}")
                        nc.vector.tensor_copy(stg[0:65, :], ctx_ps[h][0:65, :])
                        r0 = normp.tile([1, QH], f32, tag="r0")
                        nc.gpsimd.dma_start(r0[:], stg[64:65, :])
                        bcs = normp.tile([64, QH], f32, tag="bcs")
                        nc.gpsimd.partition_broadcast(bcs[:], r0[:])
                        bc = normp.tile([64, QH], f32, tag="bc")
                        scr = normp.tile([64, QH], f32, tag="scr")
                        nc.vector.reciprocal_approx_accurate(
                            bc[:], bcs[:], scratch=scr[:])
                        nc.vector.tensor_mul(
                            out=ctxT[hp:hp + 64, q0:q0 + QH],
                            in0=stg[0:64, :], in1=bc[:])

                # ---- out projection: out[s0+st*128 ...] = ctx @ Wo_slice ----
                for st in range(S // 128):
                    for c in range(D // CH):
                        ps = psP.tile([128, CH], f32, tag="pp")
                        nc.tensor.matmul(ps[:],
                                         ctxT[:, st * 128:(st + 1) * 128],
                                         wo_t[:, c * CH:(c + 1) * CH])
                        ot = ostp.tile([128, CH], f32, tag="ost")
                        nc.vector.tensor_copy(ot[:], ps[:])
                        nc.scalar.dma_start(
                            out_d[s0 + st * 128:s0 + (st + 1) * 128,
                                  c * CH:(c + 1) * CH], ot[:])

    nc.compile()
    return nc


def _get_nc():
    if "nc" not in _cache:
        _cache["nc"] = _build()
    return _cache["nc"]


def kernel(x, Wq, bq, Wk, bk, Wv, bv, Wo, bo):
    from concourse.bass_utils import run_bass_kernel_spmd

    nc = _get_nc()

    x = np.ascontiguousarray(np.asarray(x, dtype=np.float32))
    xt = np.ascontiguousarray(x.reshape(B * S, D).T)          # [D, B*S]
    idt = np.eye(128, dtype=np.float32)

    in_maps = []
    for c in range(NCORES):
        sl = slice(c * HSLICE, (c + 1) * HSLICE)
        in_maps.append({
            "xt": xt.astype(np.float16),
            "wq": (np.ascontiguousarray(np.asarray(Wq, np.float32)[:, sl]) / 8.0).astype(np.float16),
            "wk": np.ascontiguousarray(np.asarray(Wk, np.float32)[:, sl]).astype(np.float16),
            "wv": np.ascontiguousarray(np.asarray(Wv, np.float32)[:, sl]).astype(np.float16),
            "bq": (np.asarray(bq, np.float32)[sl] / 8.0).reshape(HSLICE, 1),
            "bk": np.asarray(bk, np.float32)[sl].reshape(HSLICE, 1),
            "bv": np.asarray(bv, np.float32)[sl].reshape(HSLICE, 1),
            "wo": np.ascontiguousarray(np.asarray(Wo, np.float32)[sl, :]),
            "idt": idt,
            "ones": np.ones((128, 1), np.float16),
        })

    res = run_bass_kernel_spmd(nc, in_maps, core_ids=list(range(NCORES)),
                               trace=bool(int(os.environ.get("KTRACE", "0"))))
    _cache["last_result"] = res
    acc = res.results[0]["out"].astype(np.float32)
    for c in range(1, NCORES):
        acc += res.results[c]["out"]
    acc += np.asarray(bo, np.float32)[None, :]
    return acc.reshape(B, S, D)



# revision 18
# speedup vs baseline: 1.4287x; 1.4287x over previous
"""Multi-head attention forward on 8 TRN2 NeuronCores — v2.

Problem: x[2,2048,1024] @ {Wq,Wk,Wv}[1024,1024] (+bias) -> 16 heads of 64,
softmax(QK^T/8)V per head, concat -> @Wo[1024,1024] + bo.

Sharding: tensor-parallel over d_hid. Core c owns 2 heads (128 dims).
Host sums the 8 partial out projections and adds bo.

v2 design vs v1 (296788ns baseline):
  - Act engine is the wall (128 exps of [128,1024] ~ 165us): everything
    else is scheduled to hide under it via an explicit global slot
    schedule (one slot per (batch, qchunk, ktile) score+exp step).
  - scores: two K=64 matmuls per (qc, ki) (no zero-padded Q tiles).
  - ctx: fp16 [65, 512] matmuls per (ki, head); the 65th weight column
    is ones so psum row 64 accumulates the softmax denominator.
    (fp8 DoubleRow was 2x faster on paper but walrus only accepts DR
    weights that are contiguous [K, 2, M] with M in {32,64,128} at
    column position 0 — no room for the denominator row, and a
    separate den matmul needs 2 PSUM banks we don't have.)
  - out projection reads on-device-normalized ctxT (f16), partials out
    in f16; the 8-way partial sum + bo stays on host.
  - all DMA issuance on sync/gpsimd queues (Act queue = exps only).
  - x^T loaded once for both batches (8MB SBUF resident).
  - PSUM: sc[128,1024]x2 (4 banks) + ctx [65,512]x2 (2) + pp ring (2).
"""

import os
import numpy as np

B, S, D = 2, 2048, 1024
NCORES = 8
HSLICE = D // NCORES          # 128 = 2 heads x 64
KT_PROJ = 8                   # d_in contraction tiles for projections
QH = 512                      # q chunk
NQC = S // QH                 # 4 q chunks per batch
NKT = S // 128                # 16 k tiles per batch
NPAIR = NKT // 2              # 8 ki pairs (fp8 DoubleRow)

_cache = {}


def _build():
    import concourse.bacc as bacc
    import concourse.tile as tile
    from concourse import mybir

    f32 = mybir.dt.float32
    f16 = mybir.dt.float16
    AF = mybir.ActivationFunctionType

    nc = bacc.Bacc("TRN2", target_bir_lowering=False, debug=False,
                   num_devices=NCORES)

    xt_d = nc.dram_tensor("xt", [D, B * S], f16, kind="ExternalInput").ap()
    wq_d = nc.dram_tensor("wq", [D, HSLICE], f16, kind="ExternalInput").ap()
    wk_d = nc.dram_tensor("wk", [D, HSLICE], f16, kind="ExternalInput").ap()
    wv_d = nc.dram_tensor("wv", [D, HSLICE], f16, kind="ExternalInput").ap()
    bq_d = nc.dram_tensor("bq", [HSLICE, 1], f32, kind="ExternalInput").ap()
    bk_d = nc.dram_tensor("bk", [HSLICE, 1], f32, kind="ExternalInput").ap()
    bv_d = nc.dram_tensor("bv", [HSLICE, 1], f32, kind="ExternalInput").ap()
    wo_d = nc.dram_tensor("wo", [HSLICE, D], f16, kind="ExternalInput").ap()
    idt_d = nc.dram_tensor("idt", [128, 128], f16, kind="ExternalInput").ap()
    out_d = nc.dram_tensor("out", [B * S, D], f16, kind="ExternalOutput").ap()

    with tile.TileContext(nc) as tc:
        with (
            tc.tile_pool(name="wpool", bufs=1) as wpool,
            tc.tile_pool(name="xtp", bufs=1) as xtp,
            tc.tile_pool(name="qk", bufs=2) as qkp,
            tc.tile_pool(name="vap", bufs=2) as vap,
            tc.tile_pool(name="etp", bufs=6) as etp,
            tc.tile_pool(name="ctxp", bufs=2) as ctxp,
            tc.tile_pool(name="stp", bufs=2) as stp,
            tc.tile_pool(name="normp", bufs=2) as normp,
            tc.tile_pool(name="ostp", bufs=4) as ostp,
            tc.tile_pool(name="psS", bufs=2, space="PSUM") as psS,
            tc.tile_pool(name="psC", bufs=1, space="PSUM") as psC,
            tc.tile_pool(name="psP", bufs=2, space="PSUM") as psP,
        ):
            # ---- weights / constants (gpsimd queue) ----
            wq_t, wk_t, wv_t = [], [], []
            for lst, src, tag in ((wq_t, wq_d, "wq"), (wk_t, wk_d, "wk"),
                                  (wv_t, wv_d, "wv")):
                for ki in range(KT_PROJ):
                    t = wpool.tile([128, HSLICE], f16, tag=f"{tag}{ki}",
                                   name=f"{tag}{ki}")
                    nc.gpsimd.dma_start(t[:], src[ki * 128:(ki + 1) * 128, :])
                    lst.append(t)
                    if ki == 3:  # biases early (first drains need them)
                        if tag == "wq":
                            bq_t = wpool.tile([128, 1], f32, tag="bq")
                            nc.gpsimd.dma_start(bq_t[:], bq_d[:])
                        elif tag == "wk":
                            bk_t = wpool.tile([128, 1], f32, tag="bk")
                            nc.gpsimd.dma_start(bk_t[:], bk_d[:])
                        else:
                            bv_t = wpool.tile([128, 1], f32, tag="bv")
                            nc.gpsimd.dma_start(bv_t[:], bv_d[:])
            idt = wpool.tile([128, 128], f16, tag="idt")
            nc.gpsimd.dma_start(idt[:], idt_d[:])
            wo_t = wpool.tile([128, D], f16, tag="wo")
            nc.gpsimd.dma_start(wo_t[:], wo_d[:])

            # ---- x^T, both batches, loaded once (sync queue) ----
            xts = []
            for ki in range(KT_PROJ):
                t = xtp.tile([128, B * S], f16, tag=f"xt{ki}", name=f"xt{ki}")
                xts.append(t)
            for half in range(2):          # batch 0 in two half-loads
                for ki in range(KT_PROJ):
                    nc.sync.dma_start(
                        xts[ki][:, half * 1024:(half + 1) * 1024],
                        xt_d[ki * 128:(ki + 1) * 128,
                             half * 1024:(half + 1) * 1024])
            for ki in range(KT_PROJ):      # batch 1 in one go
                nc.sync.dma_start(
                    xts[ki][:, S:2 * S],
                    xt_d[ki * 128:(ki + 1) * 128, S:2 * S])

            # ---- per-batch tile state ----
            qt = [{} for _ in range(B)]     # qc -> [128, 512] f16
            kt = [{} for _ in range(B)]     # c  -> [128, 512] f16
            vt = [{} for _ in range(B)]     # c  -> [128, 512] f16
            va = [{} for _ in range(B)]     # ki -> [128, 130] f16 (V^T + ones)
            et = [{} for _ in range(B)]     # (qc, p) -> [128, 2048] f16
            ctx_ps = [{} for _ in range(B)]  # (qc, h) -> [65, 512] f32 psum
            stg = [{} for _ in range(B)]    # (qc, h) -> [65, 512] f32
            ctxT = [{} for _ in range(B)]   # qc -> [128, 512] f16

            def proj_step(b, which, c):
                """One projection chunk: 8 matmuls + DVE drain w/ bias."""
                ps = psP.tile([128, 512], f32, tag="pp", name="pp")
                w_t = {"q": wq_t, "k": wk_t, "v": wv_t}[which]
                col0 = b * S + c * 512
                for ki in range(KT_PROJ):
                    nc.tensor.matmul(ps[:], w_t[ki][:],
                                     xts[ki][:, col0:col0 + 512],
                                     start=(ki == 0), stop=(ki == KT_PROJ - 1))
                if which == "q":
                    dst = qkp.tile([128, 512], f16, tag=f"qt{c}", name=f"qt{c}")
                    qt[b][c] = dst
                    b_t = bq_t
                elif which == "k":
                    dst = qkp.tile([128, 512], f16, tag=f"kt{c}", name=f"kt{c}")
                    kt[b][c] = dst
                    b_t = bk_t
                else:
                    dst = qkp.tile([128, 512], f16, tag=f"vt{c}", name=f"vt{c}")
                    vt[b][c] = dst
                    b_t = bv_t
                nc.vector.tensor_scalar_add(dst[:], ps[:], b_t[:, 0:1])

            def vaug_step(b, p):
                """Transpose V tiles ki=2p,2p+1 into f16 [128, 130] va tiles:
                per head h a [65]-col block = 64 V^T dims + a ones column
                (psum row 64 of the ctx matmul = softmax denominator)."""
                for j in range(2):
                    ki = 2 * p + j
                    c = ki // 4
                    vat = vap.tile([128, 130], f16, tag=f"va{ki}",
                                   name=f"va{ki}")
                    va[b][ki] = vat
                    ones_v = vat[:].rearrange("p (h m) -> p h m", h=2)
                    nc.gpsimd.memset(ones_v[:, :, 64:65], 1.0)
                    tp = psP.tile([128, 128], f16, tag="pp", name="tp")
                    nc.tensor.transpose(
                        tp[:], vt[b][c][:, (ki % 4) * 128:(ki % 4 + 1) * 128],
                        idt[:])
                    src = tp[:].rearrange("p (h m) -> p h m", h=2)
                    dstv = vat[:].rearrange(
                        "p (h m) -> p h m", h=2)[:, :, 0:64]
                    nc.vector.tensor_copy(dstv, src)

            def score_step(b, qc, ki):
                sc = psS.tile([128, 1024], f32, tag="sc", name="sc")
                c, kk = ki // 4, (ki % 4) * 128
                for h in range(2):
                    nc.tensor.matmul(
                        sc[:, h * 512:(h + 1) * 512],
                        kt[b][c][h * 64:(h + 1) * 64, kk:kk + 128],
                        qt[b][qc][h * 64:(h + 1) * 64, :],
                        start=True, stop=True)
                p, j = ki // 2, ki % 2
                if j == 0:
                    et[b][(qc, p)] = etp.tile([128, 2048], f16, tag="et",
                                              name="et")
                nc.scalar.activation(
                    et[b][(qc, p)][:, j * 1024:(j + 1) * 1024], sc[:], AF.Exp)

            def ctx_step(b, qc, p):
                # et tile [128, 2048] = [j0: h0|h1, j1: h0|h1] f16.
                ett = et[b][(qc, p)]
                if p == 0:
                    for h in range(2):
                        ctx_ps[b][(qc, h)] = psC.tile([65, 512], f32,
                                                      tag=f"c{h}", name=f"c{h}")
                for j in range(2):
                    ki = 2 * p + j
                    for h in range(2):
                        nc.tensor.matmul(
                            ctx_ps[b][(qc, h)][:],
                            va[b][ki][:, h * 65:(h + 1) * 65],
                            ett[:, j * 1024 + h * 512:j * 1024 + (h + 1) * 512],
                            start=(ki == 0), stop=(ki == NKT - 1))

            def stage_step(b, qc):
                """Drain ctx psum (frees psC fast) + kick off denom path."""
                for h in range(2):
                    st = stp.tile([65, 512], f32, tag=f"st{h}", name=f"st{h}")
                    stg[b][(qc, h)] = st
                    nc.vector.tensor_copy(st[:], ctx_ps[b][(qc, h)][0:65, :])

            def norm_step(b, qc):
                t = ctxp.tile([128, 512], f16, tag=f"ctxT{qc}",
                              name=f"ctxT{qc}")
                ctxT[b][qc] = t
                for h in range(2):
                    st = stg[b][(qc, h)]
                    r0 = normp.tile([1, 512], f32, tag=f"r0{h}", name=f"r0{h}")
                    nc.gpsimd.dma_start(r0[:], st[64:65, :])
                    rc = normp.tile([1, 512], f32, tag=f"rc{h}", name=f"rc{h}")
                    nc.vector.reciprocal_approx_fast(rc[:], r0[:])
                    bc = normp.tile([64, 512], f32, tag=f"bc{h}", name=f"bc{h}")
                    nc.gpsimd.partition_broadcast(bc[:], rc[:])
                    nc.vector.tensor_mul(
                        out=t[h * 64:(h + 1) * 64, :],
                        in0=st[0:64, :], in1=bc[:])

            def outp_step(b, qc, st_i):
                row0 = b * S + qc * 512 + st_i * 128
                for half in range(2):
                    po = psP.tile([128, 512], f32, tag="pp", name="po")
                    nc.tensor.matmul(
                        po[:],
                        ctxT[b][qc][:, st_i * 128:(st_i + 1) * 128],
                        wo_t[:, half * 512:(half + 1) * 512],
                        start=True, stop=True)
                    ot = ostp.tile([128, 512], f16, tag="ost", name="ost")
                    nc.vector.tensor_copy(ot[:], po[:])
                    eng = nc.sync if half == 0 else nc.gpsimd
                    eng.dma_start(
                        out_d[row0:row0 + 128, half * 512:(half + 1) * 512],
                        ot[:])

            # ---- global slot schedule ----
            from collections import defaultdict
            actions = defaultdict(list)   # g -> [(prio, fn)]

            # slot priorities: scores(0) feed the Act engine (the wall);
            # stage(1) frees ctx psum BEFORE the next qchunk's first ctx
            # allocates it (prio 3 > 1 at the shared slot); then outp(4)
            # and proj/vaug fillers(5).
            for b in range(B):
                base = b * 64
                for qc in range(NQC):
                    for ki in range(NKT):
                        g = base + qc * 16 + ki
                        actions[g].append(
                            (0, (lambda b=b, qc=qc, ki=ki:
                                 score_step(b, qc, ki))))
                    for p in range(NPAIR):
                        g = base + qc * 16 + 6 + 2 * p
                        actions[g].append(
                            (3, (lambda b=b, qc=qc, p=p: ctx_step(b, qc, p))))
                    actions[base + qc * 16 + 21].append(
                        (1, (lambda b=b, qc=qc: stage_step(b, qc))))
                    actions[base + qc * 16 + 23].append(
                        (2, (lambda b=b, qc=qc: norm_step(b, qc))))
                    for st_i in range(4):
                        g = base + qc * 16 + 28 + 2 * st_i
                        actions[g].append(
                            (4, (lambda b=b, qc=qc, s=st_i:
                                 outp_step(b, qc, s))))

            # proj/vaug fillers: batch 0 prologue runs before slot 0; the
            # rest interleave into earlier slots at priority 5.
            def F(step, *a):
                return lambda: step(*a)

            fill0 = {0: F(proj_step, 0, "k", 1), 1: F(proj_step, 0, "k", 2),
                     2: F(vaug_step, 0, 1), 3: F(proj_step, 0, "v", 1),
                     4: F(vaug_step, 0, 2), 5: F(proj_step, 0, "k", 3),
                     6: F(vaug_step, 0, 3), 7: F(proj_step, 0, "v", 2),
                     8: F(vaug_step, 0, 4), 9: F(proj_step, 0, "q", 1),
                     10: F(vaug_step, 0, 5), 11: F(proj_step, 0, "v", 3),
                     12: F(vaug_step, 0, 6), 13: F(vaug_step, 0, 7),
                     14: F(proj_step, 0, "q", 2), 15: F(proj_step, 0, "q", 3)}
            fill1 = {44: F(proj_step, 1, "q", 0), 46: F(proj_step, 1, "k", 0),
                     48: F(proj_step, 1, "v", 0), 50: F(vaug_step, 1, 0),
                     52: F(proj_step, 1, "k", 1), 53: F(proj_step, 1, "k", 2),
                     54: F(vaug_step, 1, 1), 55: F(proj_step, 1, "v", 1),
                     56: F(vaug_step, 1, 2), 57: F(proj_step, 1, "k", 3),
                     58: F(vaug_step, 1, 3), 59: F(proj_step, 1, "v", 2),
                     60: F(vaug_step, 1, 4), 61: F(proj_step, 1, "q", 1),
                     62: F(vaug_step, 1, 5), 63: F(proj_step, 1, "v", 3),
                     64: F(vaug_step, 1, 6), 65: F(vaug_step, 1, 7),
                     66: F(proj_step, 1, "q", 2), 67: F(proj_step, 1, "q", 3)}
            for g, fn in list(fill0.items()) + list(fill1.items()):
                actions[g].append((5, fn))

            # ---- emit: prologue then slots in order ----
            proj_step(0, "q", 0)
            proj_step(0, "k", 0)
            proj_step(0, "v", 0)
            vaug_step(0, 0)
            for g in range(max(actions) + 1):
                for _, fn in sorted(actions[g], key=lambda x: x[0]):
                    fn()

    nc.compile()
    return nc


def _get_nc():
    if "nc" not in _cache:
        _cache["nc"] = _build()
    return _cache["nc"]


def kernel(x, Wq, bq, Wk, bk, Wv, bv, Wo, bo):
    from concourse.bass_utils import run_bass_kernel_spmd

    nc = _get_nc()

    x = np.ascontiguousarray(np.asarray(x, dtype=np.float32))
    xt = np.ascontiguousarray(x.reshape(B * S, D).T)          # [D, B*S]
    idt = np.eye(128, dtype=np.float16)

    in_maps = []
    for c in range(NCORES):
        sl = slice(c * HSLICE, (c + 1) * HSLICE)
        in_maps.append({
            "xt": xt.astype(np.float16),
            "wq": (np.ascontiguousarray(np.asarray(Wq, np.float32)[:, sl]) / 8.0).astype(np.float16),
            "wk": np.ascontiguousarray(np.asarray(Wk, np.float32)[:, sl]).astype(np.float16),
            "wv": np.ascontiguousarray(np.asarray(Wv, np.float32)[:, sl]).astype(np.float16),
            "bq": (np.asarray(bq, np.float32)[sl] / 8.0).reshape(HSLICE, 1),
            "bk": np.asarray(bk, np.float32)[sl].reshape(HSLICE, 1),
            "bv": np.asarray(bv, np.float32)[sl].reshape(HSLICE, 1),
            "wo": np.ascontiguousarray(np.asarray(Wo, np.float32)[sl, :]).astype(np.float16),
            "idt": idt,
        })

    res = run_bass_kernel_spmd(nc, in_maps, core_ids=list(range(NCORES)),
                               trace=bool(int(os.environ.get("KTRACE", "0"))))
    _cache["last_result"] = res
    acc = res.results[0]["out"].astype(np.float32)
    for c in range(1, NCORES):
        acc += res.results[c]["out"].astype(np.float32)
    acc += np.asarray(bo, np.float32)[None, :]
    return acc.reshape(B, S, D)


# revision 24
# speedup vs baseline: 1.4952x; 1.0465x over previous
"""Multi-head attention forward on 8 TRN2 NeuronCores — v2.

Problem: x[2,2048,1024] @ {Wq,Wk,Wv}[1024,1024] (+bias) -> 16 heads of 64,
softmax(QK^T/8)V per head, concat -> @Wo[1024,1024] + bo.

Sharding: tensor-parallel over d_hid. Core c owns 2 heads (128 dims).
Host sums the 8 partial out projections and adds bo.

v2 design vs v1 (296788ns baseline):
  - Act engine is the wall (128 exps of [128,1024] ~ 165us): everything
    else is scheduled to hide under it via an explicit global slot
    schedule (one slot per (batch, qchunk, ktile) score+exp step).
  - scores: two K=64 matmuls per (qc, ki) (no zero-padded Q tiles).
  - ctx: fp16 [65, 512] matmuls per (ki, head); the 65th weight column
    is ones so psum row 64 accumulates the softmax denominator.
    (fp8 DoubleRow was 2x faster on paper but walrus only accepts DR
    weights that are contiguous [K, 2, M] with M in {32,64,128} at
    column position 0 — no room for the denominator row, and a
    separate den matmul needs 2 PSUM banks we don't have.)
  - out projection reads on-device-normalized ctxT (f16), partials out
    in f16; the 8-way partial sum + bo stays on host.
  - all DMA issuance on sync/gpsimd queues (Act queue = exps only).
  - x^T loaded once for both batches (8MB SBUF resident).
  - PSUM: sc[128,1024]x2 (4 banks) + ctx [65,512]x2 (2) + pp ring (2).
"""

import os
import numpy as np

B, S, D = 2, 2048, 1024
NCORES = 8
HSLICE = D // NCORES          # 128 = 2 heads x 64
KT_PROJ = 8                   # d_in contraction tiles for projections
QH = 512                      # q chunk
NQC = S // QH                 # 4 q chunks per batch
NKT = S // 128                # 16 k tiles per batch
NPAIR = NKT // 2              # 8 ki pairs (fp8 DoubleRow)

_cache = {}


def _build():
    import concourse.bacc as bacc
    import concourse.tile as tile
    from concourse import mybir

    f32 = mybir.dt.float32
    f16 = mybir.dt.float16
    AF = mybir.ActivationFunctionType

    nc = bacc.Bacc("TRN2", target_bir_lowering=False, debug=False,
                   num_devices=NCORES)

    xt_d = nc.dram_tensor("xt", [D, B * S], f16, kind="ExternalInput").ap()
    wq_d = nc.dram_tensor("wq", [D, HSLICE], f16, kind="ExternalInput").ap()
    wk_d = nc.dram_tensor("wk", [D, HSLICE], f16, kind="ExternalInput").ap()
    wv_d = nc.dram_tensor("wv", [D, HSLICE], f16, kind="ExternalInput").ap()
    bq_d = nc.dram_tensor("bq", [HSLICE, 1], f32, kind="ExternalInput").ap()
    bk_d = nc.dram_tensor("bk", [HSLICE, 1], f32, kind="ExternalInput").ap()
    bv_d = nc.dram_tensor("bv", [HSLICE, 1], f32, kind="ExternalInput").ap()
    wo_d = nc.dram_tensor("wo", [HSLICE, D], f16, kind="ExternalInput").ap()
    idt_d = nc.dram_tensor("idt", [128, 128], f16, kind="ExternalInput").ap()
    out_d = nc.dram_tensor("out", [B * S, D], f16, kind="ExternalOutput").ap()

    with tile.TileContext(nc) as tc:
        with (
            tc.tile_pool(name="wpool", bufs=1) as wpool,
            tc.tile_pool(name="xtp", bufs=1) as xtp,
            tc.tile_pool(name="qk", bufs=2) as qkp,
            tc.tile_pool(name="vap", bufs=2) as vap,
            tc.tile_pool(name="etp", bufs=6) as etp,
            tc.tile_pool(name="ctxp", bufs=2) as ctxp,
            tc.tile_pool(name="stp", bufs=2) as stp,
            tc.tile_pool(name="normp", bufs=2) as normp,
            tc.tile_pool(name="ostp", bufs=4) as ostp,
            tc.tile_pool(name="psS", bufs=2, space="PSUM") as psS,
            tc.tile_pool(name="psC", bufs=1, space="PSUM") as psC,
            tc.tile_pool(name="psP", bufs=2, space="PSUM") as psP,
        ):
            # ---- weights / constants, split across idle queues so the
            # K-projection isn't gated behind 19us of serial descriptor
            # issue. gpsimd: wq, idt, wo, then wv; scalar: wk only (9
            # descriptors, done by ~13us — before the first ACTIVATE). ----
            def wtiles(tag, src, eng, bias_d, bias_tag):
                lst, b_t = [], None
                for ki in range(KT_PROJ):
                    t = wpool.tile([128, HSLICE], f16, tag=f"{tag}{ki}",
                                   name=f"{tag}{ki}")
                    eng.dma_start(t[:], src[ki * 128:(ki + 1) * 128, :])
                    lst.append(t)
                    if ki == 3:
                        b_t = wpool.tile([128, 1], f32, tag=bias_tag,
                                         name=bias_tag)
                        eng.dma_start(b_t[:], bias_d[:])
                return lst, b_t

            wq_t, bq_t = wtiles("wq", wq_d, nc.gpsimd, bq_d, "bq")
            wk_t, bk_t = wtiles("wk", wk_d, nc.scalar, bk_d, "bk")
            idt = wpool.tile([128, 128], f16, tag="idt")
            nc.gpsimd.dma_start(idt[:], idt_d[:])
            wo_t = wpool.tile([128, D], f16, tag="wo")
            nc.gpsimd.dma_start(wo_t[:], wo_d[:])
            wv_t, bv_t = wtiles("wv", wv_d, nc.gpsimd, bv_d, "bv")

            # ---- x^T, both batches, loaded once (sync queue) ----
            xts = []
            for ki in range(KT_PROJ):
                t = xtp.tile([128, B * S], f16, tag=f"xt{ki}", name=f"xt{ki}")
                xts.append(t)
            for half in range(2):          # batch 0 in two half-loads
                for ki in range(KT_PROJ):
                    nc.sync.dma_start(
                        xts[ki][:, half * 1024:(half + 1) * 1024],
                        xt_d[ki * 128:(ki + 1) * 128,
                             half * 1024:(half + 1) * 1024])
            for ki in range(KT_PROJ):      # batch 1 in one go
                nc.sync.dma_start(
                    xts[ki][:, S:2 * S],
                    xt_d[ki * 128:(ki + 1) * 128, S:2 * S])

            # ---- per-batch tile state ----
            qt = [{} for _ in range(B)]     # qc -> [128, 512] f16
            kt = [{} for _ in range(B)]     # c  -> [128, 512] f16
            vt = [{} for _ in range(B)]     # c  -> [128, 512] f16
            va = [{} for _ in range(B)]     # ki -> [128, 130] f16 (V^T + ones)
            et = [{} for _ in range(B)]     # (qc, p) -> [128, 2048] f16
            ctx_ps = [{} for _ in range(B)]  # (qc, h) -> [65, 512] f32 psum
            stg = [{} for _ in range(B)]    # (qc, h) -> [65, 512] f32
            ctxT = [{} for _ in range(B)]   # qc -> [128, 512] f16

            def proj_step(b, which, c):
                """One projection chunk: 8 matmuls + DVE drain w/ bias."""
                ps = psP.tile([128, 512], f32, tag="pp", name="pp")
                w_t = {"q": wq_t, "k": wk_t, "v": wv_t}[which]
                col0 = b * S + c * 512
                for ki in range(KT_PROJ):
                    nc.tensor.matmul(ps[:], w_t[ki][:],
                                     xts[ki][:, col0:col0 + 512],
                                     start=(ki == 0), stop=(ki == KT_PROJ - 1))
                if which == "q":
                    dst = qkp.tile([128, 512], f16, tag=f"qt{c}", name=f"qt{c}")
                    qt[b][c] = dst
                    b_t = bq_t
                elif which == "k":
                    dst = qkp.tile([128, 512], f16, tag=f"kt{c}", name=f"kt{c}")
                    kt[b][c] = dst
                    b_t = bk_t
                else:
                    dst = qkp.tile([128, 512], f16, tag=f"vt{c}", name=f"vt{c}")
                    vt[b][c] = dst
                    b_t = bv_t
                nc.vector.tensor_scalar_add(dst[:], ps[:], b_t[:, 0:1])

            def vaug_step(b, p):
                """Transpose V tiles ki=2p,2p+1 into f16 [128, 130] va tiles:
                per head h a [65]-col block = 64 V^T dims + a ones column
                (psum row 64 of the ctx matmul = softmax denominator)."""
                for j in range(2):
                    ki = 2 * p + j
                    c = ki // 4
                    vat = vap.tile([128, 130], f16, tag=f"va{ki}",
                                   name=f"va{ki}")
                    va[b][ki] = vat
                    ones_v = vat[:].rearrange("p (h m) -> p h m", h=2)
                    nc.gpsimd.memset(ones_v[:, :, 64:65], 1.0)
                    tp = psP.tile([128, 128], f16, tag="pp", name="tp")
                    nc.tensor.transpose(
                        tp[:], vt[b][c][:, (ki % 4) * 128:(ki % 4 + 1) * 128],
                        idt[:])
                    src = tp[:].rearrange("p (h m) -> p h m", h=2)
                    dstv = vat[:].rearrange(
                        "p (h m) -> p h m", h=2)[:, :, 0:64]
                    nc.vector.tensor_copy(dstv, src)

            def score_step(b, qc, ki):
                sc = psS.tile([128, 1024], f32, tag="sc", name="sc")
                c, kk = ki // 4, (ki % 4) * 128
                for h in range(2):
                    nc.tensor.matmul(
                        sc[:, h * 512:(h + 1) * 512],
                        kt[b][c][h * 64:(h + 1) * 64, kk:kk + 128],
                        qt[b][qc][h * 64:(h + 1) * 64, :],
                        start=True, stop=True)
                p, j = ki // 2, ki % 2
                if j == 0:
                    et[b][(qc, p)] = etp.tile([128, 2048], f16, tag="et",
                                              name="et")
                nc.scalar.activation(
                    et[b][(qc, p)][:, j * 1024:(j + 1) * 1024], sc[:], AF.Exp)

            def ctx_step(b, qc, p):
                # et tile [128, 2048] = [j0: h0|h1, j1: h0|h1] f16.
                ett = et[b][(qc, p)]
                if p == 0:
                    for h in range(2):
                        ctx_ps[b][(qc, h)] = psC.tile([65, 512], f32,
                                                      tag=f"c{h}", name=f"c{h}")
                for j in range(2):
                    ki = 2 * p + j
                    for h in range(2):
                        nc.tensor.matmul(
                            ctx_ps[b][(qc, h)][:],
                            va[b][ki][:, h * 65:(h + 1) * 65],
                            ett[:, j * 1024 + h * 512:j * 1024 + (h + 1) * 512],
                            start=(ki == 0), stop=(ki == NKT - 1))

            def stage_step(b, qc):
                """Drain ctx psum (frees psC fast) + kick off denom path."""
                for h in range(2):
                    st = stp.tile([65, 512], f32, tag=f"st{h}", name=f"st{h}")
                    stg[b][(qc, h)] = st
                    nc.vector.tensor_copy(st[:], ctx_ps[b][(qc, h)][0:65, :])

            bc_t = [{} for _ in range(B)]   # (qc, h) -> [64, 512] f32

            def normA_step(b, qc):
                """Reciprocal + broadcast of the denominators. Split from
                the muls so the DVE's in-order queue never sits waiting on
                the gpsimd broadcast round-trip (that stall delayed outp
                drains and showed up as 7us PE psum-ring waits)."""
                for h in range(2):
                    st = stg[b][(qc, h)]
                    r0 = normp.tile([1, 512], f32, tag=f"r0{h}", name=f"r0{h}")
                    nc.gpsimd.dma_start(r0[:], st[64:65, :])
                    rc = normp.tile([1, 512], f32, tag=f"rc{h}", name=f"rc{h}")
                    nc.vector.reciprocal_approx_fast(rc[:], r0[:])
                    bc = normp.tile([64, 512], f32, tag=f"bc{h}", name=f"bc{h}")
                    nc.gpsimd.partition_broadcast(bc[:], rc[:])
                    bc_t[b][(qc, h)] = bc

            def normB_step(b, qc):
                t = ctxp.tile([128, 512], f16, tag=f"ctxT{qc}",
                              name=f"ctxT{qc}")
                ctxT[b][qc] = t
                for h in range(2):
                    nc.vector.tensor_mul(
                        out=t[h * 64:(h + 1) * 64, :],
                        in0=stg[b][(qc, h)][0:64, :], in1=bc_t[b][(qc, h)][:])

            def outp_step(b, qc, st_i):
                row0 = b * S + qc * 512 + st_i * 128
                for half in range(2):
                    po = psP.tile([128, 512], f32, tag="pp", name="po")
                    nc.tensor.matmul(
                        po[:],
                        ctxT[b][qc][:, st_i * 128:(st_i + 1) * 128],
                        wo_t[:, half * 512:(half + 1) * 512],
                        start=True, stop=True)
                    ot = ostp.tile([128, 512], f16, tag="ost", name="ost")
                    nc.vector.tensor_copy(ot[:], po[:])
                    eng = nc.sync if half == 0 else nc.gpsimd
                    eng.dma_start(
                        out_d[row0:row0 + 128, half * 512:(half + 1) * 512],
                        ot[:])

            # ---- global slot schedule ----
            from collections import defaultdict
            actions = defaultdict(list)   # g -> [(prio, fn)]

            # slot priorities: scores(0) feed the Act engine (the wall);
            # stage(1) frees ctx psum BEFORE the next qchunk's first ctx
            # allocates it (prio 3 > 1 at the shared slot); then outp(4)
            # and proj/vaug fillers(5).
            for b in range(B):
                base = b * 64
                for qc in range(NQC):
                    tail = (b == B - 1 and qc == NQC - 1)
                    for ki in range(NKT):
                        g = base + qc * 16 + ki
                        actions[g].append(
                            (0, (lambda b=b, qc=qc, ki=ki:
                                 score_step(b, qc, ki))))
                    for p in range(NPAIR):
                        # compress the very last qchunk: no exps pace the
                        # tail, and HAM tends to run it at half speed.
                        off = 6 + 2 * p if not (tail and p >= 6) else 11 + p
                        actions[base + qc * 16 + off].append(
                            (3, (lambda b=b, qc=qc, p=p: ctx_step(b, qc, p))))
                    o_st, o_nA, o_nB, o_out = (
                        (21, 23, 25, 28) if not tail else (19, 20, 21, 22))
                    actions[base + qc * 16 + o_st].append(
                        (1, (lambda b=b, qc=qc: stage_step(b, qc))))
                    actions[base + qc * 16 + o_nA].append(
                        (2, (lambda b=b, qc=qc: normA_step(b, qc))))
                    actions[base + qc * 16 + o_nB].append(
                        (2, (lambda b=b, qc=qc: normB_step(b, qc))))
                    for st_i in range(4):
                        g = base + qc * 16 + o_out + (2 if not tail else 1) * st_i
                        actions[g].append(
                            (4, (lambda b=b, qc=qc, s=st_i:
                                 outp_step(b, qc, s))))

            # proj/vaug fillers: batch 0 prologue runs before slot 0; the
            # rest interleave into earlier slots at priority 5.
            def F(step, *a):
                return lambda: step(*a)

            fill0 = {0: F(proj_step, 0, "k", 1), 1: F(proj_step, 0, "k", 2),
                     2: F(vaug_step, 0, 1), 3: F(proj_step, 0, "v", 1),
                     4: F(vaug_step, 0, 2), 5: F(proj_step, 0, "k", 3),
                     6: F(vaug_step, 0, 3), 7: F(proj_step, 0, "v", 2),
                     8: F(vaug_step, 0, 4), 9: F(proj_step, 0, "q", 1),
                     10: F(vaug_step, 0, 5), 11: F(proj_step, 0, "v", 3),
                     12: F(vaug_step, 0, 6), 13: F(vaug_step, 0, 7),
                     14: F(proj_step, 0, "q", 2), 15: F(proj_step, 0, "q", 3)}
            fill1 = {44: F(proj_step, 1, "q", 0), 46: F(proj_step, 1, "k", 0),
                     48: F(proj_step, 1, "v", 0), 50: F(vaug_step, 1, 0),
                     52: F(proj_step, 1, "k", 1), 53: F(proj_step, 1, "k", 2),
                     54: F(vaug_step, 1, 1), 55: F(proj_step, 1, "v", 1),
                     56: F(vaug_step, 1, 2), 57: F(proj_step, 1, "k", 3),
                     58: F(vaug_step, 1, 3), 59: F(proj_step, 1, "v", 2),
                     60: F(vaug_step, 1, 4), 61: F(proj_step, 1, "q", 1),
                     62: F(vaug_step, 1, 5), 63: F(proj_step, 1, "v", 3),
                     64: F(vaug_step, 1, 6), 65: F(vaug_step, 1, 7),
                     66: F(proj_step, 1, "q", 2), 67: F(proj_step, 1, "q", 3)}
            for g, fn in list(fill0.items()) + list(fill1.items()):
                actions[g].append((5, fn))

            # ---- emit: prologue then slots in order ----
            proj_step(0, "q", 0)
            proj_step(0, "k", 0)
            proj_step(0, "v", 0)
            vaug_step(0, 0)
            for g in range(max(actions) + 1):
                for _, fn in sorted(actions[g], key=lambda x: x[0]):
                    fn()

    nc.compile()
    return nc


def _get_nc():
    if "nc" not in _cache:
        _cache["nc"] = _build()
    return _cache["nc"]


def kernel(x, Wq, bq, Wk, bk, Wv, bv, Wo, bo):
    from concourse.bass_utils import run_bass_kernel_spmd

    nc = _get_nc()

    x = np.ascontiguousarray(np.asarray(x, dtype=np.float32))
    xt = np.ascontiguousarray(x.reshape(B * S, D).T)          # [D, B*S]
    idt = np.eye(128, dtype=np.float16)

    in_maps = []
    for c in range(NCORES):
        sl = slice(c * HSLICE, (c + 1) * HSLICE)
        in_maps.append({
            "xt": xt.astype(np.float16),
            "wq": (np.ascontiguousarray(np.asarray(Wq, np.float32)[:, sl]) / 8.0).astype(np.float16),
            "wk": np.ascontiguousarray(np.asarray(Wk, np.float32)[:, sl]).astype(np.float16),
            "wv": np.ascontiguousarray(np.asarray(Wv, np.float32)[:, sl]).astype(np.float16),
            "bq": (np.asarray(bq, np.float32)[sl] / 8.0).reshape(HSLICE, 1),
            "bk": np.asarray(bk, np.float32)[sl].reshape(HSLICE, 1),
            "bv": np.asarray(bv, np.float32)[sl].reshape(HSLICE, 1),
            "wo": np.ascontiguousarray(np.asarray(Wo, np.float32)[sl, :]).astype(np.float16),
            "idt": idt,
        })

    res = run_bass_kernel_spmd(nc, in_maps, core_ids=list(range(NCORES)),
                               trace=bool(int(os.environ.get("KTRACE", "0"))))
    _cache["last_result"] = res
    acc = res.results[0]["out"].astype(np.float32)
    for c in range(1, NCORES):
        acc += res.results[c]["out"].astype(np.float32)
    acc += np.asarray(bo, np.float32)[None, :]
    return acc.reshape(B, S, D)
